# revision 55
# baseline (speedup 1.0000x reference)
"""CustomGaussianLayer Trainium2 kernel.

Math: out[b,o] = sum_{i,g} exp(-0.5*((tanh(x[b,i])-c_g)/w)^2) * coeff[o,i,g]*W[o,i]
 == E @ W2T  with  E[b, k=(g,i)] Gaussian basis,  W2T[k, o] folded weights.

Reference-center factorization (c4 = centers[4] = 1/7, smallest |c|):
  E_4 = exp(-a*(t-c4)^2)            directly: (t-c4)^2 -> Exp(-a*.)
  E_g = E_4 * exp(2a*(c_g-c4)*t)    for g != 4 (one ACT exp + one DVE mult)
Host folds exp(-a*(c_g^2-c4^2)) <= 1 into the weights (no overflow: c4 is
the smallest center), replacing the exp(-a t^2)*exp(2a c_g t) pair per g.

Per core (data-parallel over batch, 1024 rows each):
  ACT: tanh + exps (+ h1 squares) ; DVE: h0 sub/sq + E mults ; PE: f32r
  matmuls [o,b] += W2T_k^T E_k (1 cycle/row at free>=256). Startup is
  512-col fine-grained with the input DMA chain in consumer order; PE
  ramps on warm-up matmuls against a zeroed scratch from ~1us. The last
  4 k-tiles are swept (ot,bc)-outer so the 8 psum banks close staggered
  ~850ns apart; bf16 drains chase on DVE/ACT with their DMAs spread over
  Pool/SP/ACT queues so the output wire and pushes stay off the critical
  path.
"""

import numpy as np

import concourse.bacc as bacc
import concourse.mybir as mybir
import concourse.tile as tile
from concourse.bass_utils import run_bass_kernel_spmd
from concourse.tile import add_dep_helper

G = 8
I_SZ = 512
O_SZ = 512
B = 8192
NCORES = 8
B_SH = B // NCORES          # 1024 batch rows per core
K = I_SZ * G                # 4096 contraction
N_IBLK = I_SZ // 128        # 4 partition blocks of i
FREE = N_IBLK * B_SH        # 4096 free layout (i_blk, b)
HALF = FREE // 2            # 2048 (i_blk 0-1 | 2-3)
N_OT = O_SZ // 128          # 4 output tiles
N_BC = B_SH // 512          # 2 batch chunks of 512 (psum free limit fp32)

ALPHA = 24.5                # 0.5 / width^2, width = 1/7
N_WARMUP = 8
CENTERS = np.linspace(-1.0, 1.0, G).astype(np.float32)
C4 = float(CENTERS[4])      # 1/7, reference center
GIDX = [4, 0, 1, 2, 3, 5, 6, 7]   # E-tile production order -> g
# device k-tile position: pos = h*16 + gi*2 + ib_loc

F32 = mybir.dt.float32
F32R = mybir.dt.float32r
AF = mybir.ActivationFunctionType
ALU = mybir.AluOpType

_NC_CACHE = {}


def build_nc():
    nc = bacc.Bacc("TRN2", target_bir_lowering=False)
    xt_d = nc.dram_tensor("xt", [I_SZ, B_SH], F32, kind="ExternalInput")
    w2t_d = nc.dram_tensor("w2t", [K, O_SZ], F32R, kind="ExternalInput")
    out_d = nc.dram_tensor(
        "out_t", [O_SZ, B_SH], mybir.dt.bfloat16, kind="ExternalOutput")

    with tile.TileContext(nc) as tc:
        with (
            tc.tile_pool(name="w2", bufs=1) as w2_pool,
            tc.tile_pool(name="xt", bufs=1) as xt_pool,
            tc.tile_pool(name="tt", bufs=1) as tt_pool,
            tc.tile_pool(name="uu", bufs=2) as uu_pool,
            tc.tile_pool(name="qq", bufs=2) as qq_pool,
            tc.tile_pool(name="e4", bufs=2) as e4_pool,
            tc.tile_pool(name="bb", bufs=3) as bb_pool,
            tc.tile_pool(name="ee", bufs=4) as ee_pool,
            tc.tile_pool(name="wm", bufs=1) as wm_pool,
            tc.tile_pool(name="ps", bufs=1, space="PSUM") as ps_pool,
            tc.tile_pool(name="ob", bufs=8) as ob_pool,
        ):
            act_ops = []   # pinned ACT engine order
            dve_ops = []   # pinned DVE engine order
            pe_ops = []    # pinned PE engine order

            # trigger the ACT table load immediately (~1.3us; the
            # exp_and_others set serves Exp, Tanh and Square together)
            actwarm = wm_pool.tile([128, 1], F32, tag="actwarm")
            act_ops.append(nc.scalar.activation(
                actwarm[:], nc.const_aps.tensor(0.0, (128, 1)), AF.Exp,
            ))

            # zeroed scratch for PE warm-up matmuls: lets the PE p-state
            # ramp start at ~1us instead of waiting for the first w2 DMA
            warm = wm_pool.tile([128, 640], F32R, tag="warm")
            dve_ops.append(nc.vector.memset(
                warm[:].bitcast(mybir.dt.uint32), 0))
            # per-partition scalar bias (-c4) for the h1 Square activations
            c4b = wm_pool.tile([128, 1], F32, tag="c4b")
            dve_ops.append(nc.vector.memset(c4b[:], -C4))

            w2_all = w2_pool.tile([128, (K // 128) * O_SZ], F32R, tag="w2all")
            w2t_v = w2t_d[:, :].rearrange("(kt p) o -> p kt o", p=128)
            xt_sb = xt_pool.tile([128, FREE], F32, tag="xt")
            tt = tt_pool.tile([128, FREE], F32, tag="tt")
            xt_v = xt_d[:, :].rearrange("(ib p) b -> p ib b", p=128)

            def w2_dma(kt_lo, kt_hi):
                return nc.sync.dma_start(
                    w2_all[:, kt_lo * O_SZ:kt_hi * O_SZ]
                    .rearrange("p (kt o) -> p kt o", o=O_SZ),
                    w2t_v[:, kt_lo:kt_hi, :],
                )

            def xt_dma_fine(c):
                # one 512-col chunk (c in units of 512 free cols)
                return nc.sync.dma_start(
                    xt_sb[:, c * 512:(c + 1) * 512],
                    xt_v[:, c // 2, (c % 2) * 512:(c % 2 + 1) * 512],
                )

            def xt_dma_q(p):
                # one 256-col piece (p in units of 256 free cols)
                return nc.sync.dma_start(
                    xt_sb[:, p * 256:(p + 1) * 256],
                    xt_v[:, p // 4, (p % 4) * 256:(p % 4 + 1) * 256],
                )

            def xt_dma(ib_lo, ib_hi):
                return nc.sync.dma_start(
                    xt_sb[:, ib_lo * B_SH:ib_hi * B_SH]
                    .rearrange("p (ib b) -> p ib b", b=B_SH),
                    xt_v[:, ib_lo:ib_hi, :],
                )

            # SP DMA lane in consumer order; xt chunks lead so the ACT
            # tanh chain never stalls, w2 pos-pairs chase PE consumption
            dma_chain = [
                xt_dma_fine(0),        # c0: ib0 b0-511  -> tanh_c0
                xt_dma_q(2),           # c1 first half: tanh_c1a sooner
                xt_dma_q(3),           # c1 second half
                xt_dma_fine(2),        # c2
                w2_dma(0, 1),          # pos0 (h0 gi0=g4 ib0): first real mm
                xt_dma_fine(3),        # c3
                w2_dma(1, 2),          # pos1
                w2_dma(2, 4),          # pos2-3 (gi1)
                w2_dma(4, 8),          # pos4-7 (gi2-3)
                w2_dma(8, 12),         # pos8-11 (gi4-5 h0)
                xt_dma(2, 4),          # h1
                w2_dma(12, 16),        # pos12-15 (gi6-7 h0)
                w2_dma(16, 24),        # h1 gi0-3
                w2_dma(24, 32),        # h1 gi4-7
            ]

            psum = [
                [
                    ps_pool.tile(
                        [128, 512], F32,
                        name=f"ps{ot}_{bc}", tag=f"ps{ot}_{bc}",
                    )
                    for bc in range(N_BC)
                ]
                for ot in range(N_OT)
            ]

            # PE warm-up on the zeroed scratch (disjoint lhsT/rhs regions)
            for w in range(N_WARMUP):
                pe_ops.append(nc.tensor.matmul(
                    psum[0][0][:], warm[:, 0:128], warm[:, 128:640],
                    start=(w == 0), stop=(w == N_WARMUP - 1),
                ))

            # ---- basis production ---------------------------------------
            s_g = {g: float(2.0 * ALPHA * (CENTERS[g] - C4)) for g in range(G)}
            e4h = [
                e4_pool.tile([128, HALF], F32R, tag="e4", name=f"e4h{h}")
                for h in range(2)
            ]

            # h0 fine phase (512-col chunks): tanh + E4-exp on ACT,
            # (t-c4) and square on DVE; t1 split in 256-col halves so the
            # first half starts as soon as its smaller DMA lands
            t_f, e4_f = [], []
            for c in range(4):
                sl = slice(c * 512, (c + 1) * 512)
                if c == 1:
                    t_f.append([
                        nc.scalar.activation(
                            tt[:, 512:768], xt_sb[:, 512:768], AF.Tanh),
                        nc.scalar.activation(
                            tt[:, 768:1024], xt_sb[:, 768:1024], AF.Tanh),
                    ])
                else:
                    t_f.append(nc.scalar.activation(
                        tt[:, sl], xt_sb[:, sl], AF.Tanh))
                u = uu_pool.tile([128, 512], F32, tag="uf")
                q = qq_pool.tile([128, 512], F32, tag="qf")
                dve_ops.append(nc.vector.tensor_scalar_sub(u[:], tt[:, sl], C4))
                dve_ops.append(nc.vector.tensor_tensor(q[:], u[:], u[:], op=ALU.mult))
                e4_f.append(nc.scalar.activation(
                    e4h[0][:, sl], q[:], AF.Exp, scale=-ALPHA))
            # B' / E tiles for gi >= 1
            btile = {}
            etile = {}

            def b_part(h, gi, lo, hi):
                g = GIDX[gi]
                if (h, gi) not in btile:
                    btile[(h, gi)] = bb_pool.tile(
                        [128, HALF], F32, tag="bb", name=f"b_{h}_{gi}")
                bt = btile[(h, gi)]
                hsl = h * HALF
                act_ops.append(nc.scalar.activation(
                    bt[:, lo:hi], tt[:, hsl + lo:hsl + hi], AF.Exp,
                    scale=s_g[g]))

            def make_b(h, gi, parts):
                for lo, hi in parts:
                    b_part(h, gi, lo, hi)

            def make_e(h, gi, parts):
                et = ee_pool.tile([128, HALF], F32R, tag="ee",
                                  name=f"e_{h}_{gi}")
                etile[(h, gi)] = et
                bt = btile[(h, gi)]
                for lo, hi in parts:
                    dve_ops.append(nc.vector.tensor_tensor(
                        et[:, lo:hi], e4h[h][:, lo:hi], bt[:, lo:hi],
                        op=ALU.mult))

            SUB = [(0, 1024), (1024, 2048)]
            FULLP = [(0, 2048)]
            # h0 fine-phase ACT order: t0, t1, E4_0, t2, E4_1, t3, B1a,
            # E4_2, E4_3, ... — B1a needs only t0/t1 + E4_0/E4_1 downstream,
            # so hoisting it lets the gi1 feed (PE saturation anchor) start
            # ~1.3us earlier while c2/c3 stall on E4_2/E4_3 anyway
            act_ops += [t_f[0], t_f[1][0], t_f[1][1], e4_f[0], t_f[2],
                        e4_f[1], t_f[3], e4_f[2]]
            b_part(0, 1, 0, 512)                      # B1 quarter 0
            act_ops += [e4_f[3]]
            b_part(0, 1, 512, 1024)                   # B1 quarter 1
            b_part(0, 1, 1024, 2048)                  # B1b
            # h0: gi2 at 1024-col subtiles; gi3..7 full-half
            make_b(0, 2, SUB)
            for gi in range(3, 8):
                make_b(0, gi, FULLP)
            # h1 E4 chain at 1024-col parts: tanh, Square(t - c4), exp
            t_h1, e4_h1 = [], []
            q_h1 = []
            for p in range(2):
                sl = slice(HALF + p * 1024, HALF + (p + 1) * 1024)
                esl = slice(p * 1024, (p + 1) * 1024)
                t_i = nc.scalar.activation(tt[:, sl], xt_sb[:, sl], AF.Tanh)
                q = qq_pool.tile([128, 1024], F32, tag="qc")
                q_i = nc.scalar.activation(q[:], tt[:, sl], AF.Square, bias=c4b[:])
                e_i = nc.scalar.activation(
                    e4h[1][:, esl], q[:], AF.Exp, scale=-ALPHA)
                t_h1.append(t_i); q_h1.append(q_i); e4_h1.append(e_i)
            act_ops += [t_h1[0], q_h1[0], e4_h1[0],
                        t_h1[1], q_h1[1], e4_h1[1]]
            for gi in range(1, 8):
                make_b(1, gi, FULLP)

            # DVE E-mult order (matches ACT production order; gi1-ib0 at
            # 512-col quarters so the first gi1 matmuls feed right after
            # the B1 quarters land)
            make_e(0, 1, [(0, 512), (512, 1024), (1024, 2048)])
            make_e(0, 2, SUB)
            for gi in range(3, 8):
                make_e(0, gi, FULLP)
            for gi in range(1, 8):
                make_e(1, gi, FULLP)

            # ---- PE: matmuls in consumption order -----------------------
            def mm(pos, ot, bc, et, ebase, start=False, stop=False):
                lhsT = w2_all[:, pos * O_SZ + ot * 128: pos * O_SZ + (ot + 1) * 128]
                pe_ops.append(nc.tensor.matmul(
                    psum[ot][bc][:], lhsT, et[:, ebase:ebase + 512],
                    start=start, stop=stop))

            def block8(pos, et, ibl):
                for ot in range(N_OT):
                    for bc in range(N_BC):
                        mm(pos, ot, bc, et, ibl * B_SH + bc * 512)

            # h0 E4 fine chunks: chunk c -> (pos=c//2, bc=c%2)
            for c in range(4):
                for ot in range(N_OT):
                    mm(c // 2, ot, c % 2, e4h[0], c * 512, start=(c < 2))
            # h0 gi1, gi2 at subtile cadence
            for gi in (1, 2):
                for ibl in range(2):
                    block8(gi * 2 + ibl, etile[(0, gi)], ibl)
            for gi in range(3, 8):
                for ibl in range(2):
                    block8(gi * 2 + ibl, etile[(0, gi)], ibl)
            # h1: E4 parts then gi1..gi5
            for ibl in range(2):
                block8(16 + ibl, e4h[1], ibl)
            for gi in range(1, 6):
                for ibl in range(2):
                    block8(16 + gi * 2 + ibl, etile[(1, gi)], ibl)
            # tail: gi6, gi7 swept (ot, bc)-outer so the 8 banks close
            # staggered ~850ns apart and drains/DMAs chase them
            def tail_mms(ot, bc):
                for gi in (6, 7):
                    for ibl in range(2):
                        pos = 16 + gi * 2 + ibl
                        mm(pos, ot, bc, etile[(1, gi)],
                           ibl * B_SH + bc * 512,
                           stop=(gi == 7 and ibl == 1))

            # ---- drain psum -> SBUF (bf16) -> per-piece DMA out ---------
            # output pushes spread over Pool (bc0 banks), ACT (bc1 banks +
            # final piece) and SP (piece A) so the last pieces hit engines
            # that are free at closure time; bf16 halves the output wire
            pool_ops = []
            sp_out = []

            def drain(ot, bc, lo, hi, copy_eng, dma_eng):
                osb = ob_pool.tile([128, hi - lo], mybir.dt.bfloat16,
                                   tag="ob", name=f"ob{ot}_{bc}_{lo}")
                dst = out_d[ot * 128:(ot + 1) * 128,
                            bc * 512 + lo:bc * 512 + hi]
                src = psum[ot][bc][:, lo:hi]
                if copy_eng == "dve":
                    cp = nc.vector.tensor_copy(osb[:], src)
                    dve_ops.append(cp)
                else:
                    cp = nc.scalar.activation(osb[:], src, AF.Copy)
                    act_ops.append(cp)
                if dma_eng == "pool":
                    pool_ops.append(nc.gpsimd.dma_start(dst, osb[:]))
                elif dma_eng == "sp":
                    sp_out.append(nc.sync.dma_start(dst, osb[:]))
                else:
                    # branch off the copy, NOT into the act chain: a compute
                    # op pinned after a dma_start waits for the full DMA
                    # completion semaphore (~wire + 900ns)
                    dma = nc.scalar.dma_start(dst, osb[:])
                    add_dep_helper(dma.ins, cp.ins, sync=False,
                                   reason="act out dma after its copy")

            for ot in range(N_OT):
                for bc in range(N_BC):
                    tail_mms(ot, bc)
                    if bc == 0:
                        drain(ot, bc, 0, 512, "dve", "pool")
                    elif ot == N_OT - 1:
                        # last-closing bank: copy and DMA both on ACT,
                        # which is guaranteed free at closure
                        drain(ot, bc, 0, 512, "act", "act")
                    else:
                        drain(ot, bc, 0, 512, "act", "sp")

            # ---- pin engine orders --------------------------------------
            def pin(ops, reason):
                for a, b in zip(ops, ops[1:]):
                    add_dep_helper(b.ins, a.ins, sync=False, reason=reason)

            pin(act_ops, "act order")
            pin(dve_ops, "dve order")
            pin(pe_ops, "pe order")
            pin(dma_chain + sp_out, "sp dma order")
            pin(pool_ops, "pool dma order")
    nc.compile()
    return nc


def get_nc():
    if "nc" not in _NC_CACHE:
        _NC_CACHE["nc"] = build_nc()
    return _NC_CACHE["nc"]


def prep_inputs(x, weights, coefficients):
    x = np.asarray(x, dtype=np.float32)
    weights = np.asarray(weights, dtype=np.float32)
    coefficients = np.asarray(coefficients, dtype=np.float32)
    # W2T[k=(g,i), o] = coeff[o,i,g] * W[o,i] * exp(-a*(c_g^2 - c4^2))
    w2 = coefficients * weights[:, :, None]                   # [O, I, G]
    fold = np.exp(-ALPHA * (CENTERS.astype(np.float64) ** 2 - C4 * C4))
    w2t = w2.transpose(2, 1, 0).astype(np.float64) * fold[:, None, None]
    # reorder source tiles (g, ib) into device positions (h, gi, ib_loc)
    src = np.ascontiguousarray(w2t).reshape(G, N_IBLK, 128, O_SZ)
    dev = np.empty((2, G, 2, 128, O_SZ), np.float32)
    for h in range(2):
        for gi in range(G):
            for ibl in range(2):
                dev[h, gi, ibl] = src[GIDX[gi], 2 * h + ibl]
    w2t = np.ascontiguousarray(dev.reshape(K, O_SZ))
    # round-to-nearest-even to fp32r (8-bit exp, 11-bit mantissa)
    u = w2t.view(np.uint32)
    u[:] = (u + 0x7FF + ((u >> 12) & 1)) & 0xFFFFF000
    xT = np.ascontiguousarray(x.T)  # [I, B]
    in_maps = [
        {
            "xt": np.ascontiguousarray(xT[:, c * B_SH:(c + 1) * B_SH]),
            "w2t": w2t,
        }
        for c in range(NCORES)
    ]
    return in_maps


def kernel(x, weights, coefficients):
    nc = get_nc()
    in_maps = prep_inputs(x, weights, coefficients)
    res = run_bass_kernel_spmd(nc, in_maps, core_ids=list(range(NCORES)))
    out = np.empty((B, O_SZ), dtype=np.float32)
    for c in range(NCORES):
        out[c * B_SH:(c + 1) * B_SH, :] = (
            np.asarray(res.results[c]["out_t"]).astype(np.float32).T)
    return out


# revision 56
# speedup vs baseline: 1.0038x; 1.0038x over previous
"""CustomGaussianLayer Trainium2 kernel.

Math: out[b,o] = sum_{i,g} exp(-0.5*((tanh(x[b,i])-c_g)/w)^2) * coeff[o,i,g]*W[o,i]
 == E @ W2T  with  E[b, k=(g,i)] Gaussian basis,  W2T[k, o] folded weights.

Reference-center factorization (c4 = centers[4] = 1/7, smallest |c|):
  E_4 = exp(-a*(t-c4)^2)            directly: (t-c4)^2 -> Exp(-a*.)
  E_g = E_4 * exp(2a*(c_g-c4)*t)    for g != 4 (one ACT exp + one DVE mult)
Host folds exp(-a*(c_g^2-c4^2)) <= 1 into the weights (no overflow: c4 is
the smallest center), replacing the exp(-a t^2)*exp(2a c_g t) pair per g.

Per core (data-parallel over batch, 1024 rows each):
  ACT: tanh + exps (+ h1 squares) ; DVE: h0 sub/sq + E mults ; PE: f32r
  matmuls [o,b] += W2T_k^T E_k (1 cycle/row at free>=256). Startup is
  512-col fine-grained with the input DMA chain in consumer order; PE
  ramps on warm-up matmuls against a zeroed scratch from ~1us. The last
  4 k-tiles are swept (ot,bc)-outer so the 8 psum banks close staggered
  ~850ns apart; bf16 drains chase on DVE/ACT with their DMAs spread over
  Pool/SP/ACT queues so the output wire and pushes stay off the critical
  path.
"""

import numpy as np

import concourse.bacc as bacc
import concourse.mybir as mybir
import concourse.tile as tile
from concourse.bass_utils import run_bass_kernel_spmd
from concourse.tile import add_dep_helper

G = 8
I_SZ = 512
O_SZ = 512
B = 8192
NCORES = 8
B_SH = B // NCORES          # 1024 batch rows per core
K = I_SZ * G                # 4096 contraction
N_IBLK = I_SZ // 128        # 4 partition blocks of i
FREE = N_IBLK * B_SH        # 4096 free layout (i_blk, b)
HALF = FREE // 2            # 2048 (i_blk 0-1 | 2-3)
N_OT = O_SZ // 128          # 4 output tiles
N_BC = B_SH // 512          # 2 batch chunks of 512 (psum free limit fp32)

ALPHA = 24.5                # 0.5 / width^2, width = 1/7
N_WARMUP = 8
CENTERS = np.linspace(-1.0, 1.0, G).astype(np.float32)
C4 = float(CENTERS[4])      # 1/7, reference center
GIDX = [4, 0, 1, 2, 3, 5, 6, 7]   # E-tile production order -> g
# device k-tile position: pos = h*16 + gi*2 + ib_loc

F32 = mybir.dt.float32
F32R = mybir.dt.float32r
BF16 = mybir.dt.bfloat16
AF = mybir.ActivationFunctionType
ALU = mybir.AluOpType

_NC_CACHE = {}


def build_nc():
    nc = bacc.Bacc("TRN2", target_bir_lowering=False)
    xt_d = nc.dram_tensor("xt", [I_SZ, B_SH], F32, kind="ExternalInput")
    w2t_d = nc.dram_tensor("w2t", [K, O_SZ], BF16, kind="ExternalInput")
    out_d = nc.dram_tensor(
        "out_t", [O_SZ, B_SH], mybir.dt.bfloat16, kind="ExternalOutput")

    with tile.TileContext(nc) as tc:
        with (
            tc.tile_pool(name="w2", bufs=1) as w2_pool,
            tc.tile_pool(name="xt", bufs=1) as xt_pool,
            tc.tile_pool(name="tt", bufs=1) as tt_pool,
            tc.tile_pool(name="uu", bufs=2) as uu_pool,
            tc.tile_pool(name="qq", bufs=2) as qq_pool,
            tc.tile_pool(name="e4", bufs=2) as e4_pool,
            tc.tile_pool(name="bb", bufs=3) as bb_pool,
            tc.tile_pool(name="ee", bufs=4) as ee_pool,
            tc.tile_pool(name="wm", bufs=1) as wm_pool,
            tc.tile_pool(name="ps", bufs=1, space="PSUM") as ps_pool,
            tc.tile_pool(name="ob", bufs=8) as ob_pool,
        ):
            act_ops = []   # pinned ACT engine order
            dve_ops = []   # pinned DVE engine order
            pe_ops = []    # pinned PE engine order

            # trigger the ACT table load immediately (~1.3us; the
            # exp_and_others set serves Exp, Tanh and Square together)
            actwarm = wm_pool.tile([128, 1], F32, tag="actwarm")
            act_ops.append(nc.scalar.activation(
                actwarm[:], nc.const_aps.tensor(0.0, (128, 1)), AF.Exp,
            ))

            # zeroed scratch for PE warm-up matmuls: lets the PE p-state
            # ramp start at ~1us instead of waiting for the first w2 DMA
            warm = wm_pool.tile([128, 640], BF16, tag="warm")
            dve_ops.append(nc.vector.memset(
                warm[:].bitcast(mybir.dt.uint16), 0))
            # per-partition scalar bias (-c4) for the h1 Square activations
            c4b = wm_pool.tile([128, 1], F32, tag="c4b")
            dve_ops.append(nc.vector.memset(c4b[:], -C4))

            w2_all = w2_pool.tile([128, (K // 128) * O_SZ], BF16, tag="w2all")
            w2t_v = w2t_d[:, :].rearrange("(kt p) o -> p kt o", p=128)
            xt_sb = xt_pool.tile([128, FREE], F32, tag="xt")
            tt = tt_pool.tile([128, FREE], F32, tag="tt")
            xt_v = xt_d[:, :].rearrange("(ib p) b -> p ib b", p=128)

            def w2_dma(kt_lo, kt_hi):
                return nc.sync.dma_start(
                    w2_all[:, kt_lo * O_SZ:kt_hi * O_SZ]
                    .rearrange("p (kt o) -> p kt o", o=O_SZ),
                    w2t_v[:, kt_lo:kt_hi, :],
                )

            def xt_dma_fine(c):
                # one 512-col chunk (c in units of 512 free cols)
                return nc.sync.dma_start(
                    xt_sb[:, c * 512:(c + 1) * 512],
                    xt_v[:, c // 2, (c % 2) * 512:(c % 2 + 1) * 512],
                )

            def xt_dma_q(p):
                # one 256-col piece (p in units of 256 free cols)
                return nc.sync.dma_start(
                    xt_sb[:, p * 256:(p + 1) * 256],
                    xt_v[:, p // 4, (p % 4) * 256:(p % 4 + 1) * 256],
                )

            def xt_dma(ib_lo, ib_hi):
                return nc.sync.dma_start(
                    xt_sb[:, ib_lo * B_SH:ib_hi * B_SH]
                    .rearrange("p (ib b) -> p ib b", b=B_SH),
                    xt_v[:, ib_lo:ib_hi, :],
                )

            # SP DMA lane in consumer order; xt chunks lead so the ACT
            # tanh chain never stalls, w2 pos-pairs chase PE consumption
            dma_chain = [
                xt_dma_fine(0),        # c0: ib0 b0-511  -> tanh_c0
                xt_dma_q(2),           # c1 first half: tanh_c1a sooner
                xt_dma_q(3),           # c1 second half
                xt_dma_fine(2),        # c2
                w2_dma(0, 1),          # pos0 (h0 gi0=g4 ib0): first real mm
                xt_dma_fine(3),        # c3
                w2_dma(1, 2),          # pos1
                w2_dma(2, 4),          # pos2-3 (gi1)
                w2_dma(4, 8),          # pos4-7 (gi2-3)
                w2_dma(8, 12),         # pos8-11 (gi4-5 h0)
                xt_dma(2, 4),          # h1
                w2_dma(12, 16),        # pos12-15 (gi6-7 h0)
                w2_dma(16, 24),        # h1 gi0-3
                w2_dma(24, 32),        # h1 gi4-7
            ]

            psum = [
                [
                    ps_pool.tile(
                        [128, 512], F32,
                        name=f"ps{ot}_{bc}", tag=f"ps{ot}_{bc}",
                    )
                    for bc in range(N_BC)
                ]
                for ot in range(N_OT)
            ]

            # PE warm-up on the zeroed scratch (disjoint lhsT/rhs regions)
            for w in range(N_WARMUP):
                pe_ops.append(nc.tensor.matmul(
                    psum[0][0][:], warm[:, 0:128], warm[:, 128:640],
                    start=(w == 0), stop=(w == N_WARMUP - 1),
                ))

            # ---- basis production ---------------------------------------
            s_g = {g: float(2.0 * ALPHA * (CENTERS[g] - C4)) for g in range(G)}
            e4h = [
                e4_pool.tile([128, HALF], BF16, tag="e4", name=f"e4h{h}")
                for h in range(2)
            ]

            # h0 fine phase (512-col chunks): tanh + E4-exp on ACT,
            # (t-c4) and square on DVE; t1 split in 256-col halves so the
            # first half starts as soon as its smaller DMA lands
            t_f, e4_f = [], []
            for c in range(4):
                sl = slice(c * 512, (c + 1) * 512)
                if c == 1:
                    t_f.append([
                        nc.scalar.activation(
                            tt[:, 512:768], xt_sb[:, 512:768], AF.Tanh),
                        nc.scalar.activation(
                            tt[:, 768:1024], xt_sb[:, 768:1024], AF.Tanh),
                    ])
                else:
                    t_f.append(nc.scalar.activation(
                        tt[:, sl], xt_sb[:, sl], AF.Tanh))
                u = uu_pool.tile([128, 512], F32, tag="uf")
                q = qq_pool.tile([128, 512], F32, tag="qf")
                dve_ops.append(nc.vector.tensor_scalar_sub(u[:], tt[:, sl], C4))
                dve_ops.append(nc.vector.tensor_tensor(q[:], u[:], u[:], op=ALU.mult))
                e4_f.append(nc.scalar.activation(
                    e4h[0][:, sl], q[:], AF.Exp, scale=-ALPHA))
            # B' / E tiles for gi >= 1
            btile = {}
            etile = {}

            def b_part(h, gi, lo, hi):
                g = GIDX[gi]
                if (h, gi) not in btile:
                    btile[(h, gi)] = bb_pool.tile(
                        [128, HALF], BF16, tag="bb", name=f"b_{h}_{gi}")
                bt = btile[(h, gi)]
                hsl = h * HALF
                act_ops.append(nc.scalar.activation(
                    bt[:, lo:hi], tt[:, hsl + lo:hsl + hi], AF.Exp,
                    scale=s_g[g]))

            def make_b(h, gi, parts):
                for lo, hi in parts:
                    b_part(h, gi, lo, hi)

            def make_e(h, gi, parts):
                et = ee_pool.tile([128, HALF], BF16, tag="ee",
                                  name=f"e_{h}_{gi}")
                etile[(h, gi)] = et
                bt = btile[(h, gi)]
                for lo, hi in parts:
                    dve_ops.append(nc.vector.tensor_tensor(
                        et[:, lo:hi], e4h[h][:, lo:hi], bt[:, lo:hi],
                        op=ALU.mult))

            SUB = [(0, 1024), (1024, 2048)]
            FULLP = [(0, 2048)]
            # h0 fine-phase ACT order: t0, t1, E4_0, t2, E4_1, t3, B1a,
            # E4_2, E4_3, ... — B1a needs only t0/t1 + E4_0/E4_1 downstream,
            # so hoisting it lets the gi1 feed (PE saturation anchor) start
            # ~1.3us earlier while c2/c3 stall on E4_2/E4_3 anyway
            act_ops += [t_f[0], t_f[1][0], t_f[1][1], e4_f[0], t_f[2],
                        e4_f[1], t_f[3], e4_f[2]]
            b_part(0, 1, 0, 512)                      # B1 quarter 0
            act_ops += [e4_f[3]]
            b_part(0, 1, 512, 1024)                   # B1 quarter 1
            b_part(0, 1, 1024, 2048)                  # B1b
            # h0: gi2 at 1024-col subtiles; gi3..7 full-half
            make_b(0, 2, SUB)
            for gi in range(3, 8):
                make_b(0, gi, FULLP)
            # h1 E4 chain at 1024-col parts: tanh, Square(t - c4), exp
            t_h1, e4_h1 = [], []
            q_h1 = []
            for p in range(2):
                sl = slice(HALF + p * 1024, HALF + (p + 1) * 1024)
                esl = slice(p * 1024, (p + 1) * 1024)
                t_i = nc.scalar.activation(tt[:, sl], xt_sb[:, sl], AF.Tanh)
                q = qq_pool.tile([128, 1024], F32, tag="qc")
                q_i = nc.scalar.activation(q[:], tt[:, sl], AF.Square, bias=c4b[:])
                e_i = nc.scalar.activation(
                    e4h[1][:, esl], q[:], AF.Exp, scale=-ALPHA)
                t_h1.append(t_i); q_h1.append(q_i); e4_h1.append(e_i)
            act_ops += [t_h1[0], q_h1[0], e4_h1[0],
                        t_h1[1], q_h1[1], e4_h1[1]]
            for gi in range(1, 8):
                make_b(1, gi, FULLP)

            # DVE E-mult order (matches ACT production order; gi1-ib0 at
            # 512-col quarters so the first gi1 matmuls feed right after
            # the B1 quarters land)
            make_e(0, 1, [(0, 512), (512, 1024), (1024, 2048)])
            make_e(0, 2, SUB)
            for gi in range(3, 8):
                make_e(0, gi, FULLP)
            for gi in range(1, 8):
                make_e(1, gi, FULLP)

            # ---- PE: matmuls in consumption order -----------------------
            def mm(pos, ot, bc, et, ebase, start=False, stop=False):
                lhsT = w2_all[:, pos * O_SZ + ot * 128: pos * O_SZ + (ot + 1) * 128]
                pe_ops.append(nc.tensor.matmul(
                    psum[ot][bc][:], lhsT, et[:, ebase:ebase + 512],
                    start=start, stop=stop))

            def block8(pos, et, ibl):
                for ot in range(N_OT):
                    for bc in range(N_BC):
                        mm(pos, ot, bc, et, ibl * B_SH + bc * 512)

            # h0 E4 fine chunks: chunk c -> (pos=c//2, bc=c%2)
            for c in range(4):
                for ot in range(N_OT):
                    mm(c // 2, ot, c % 2, e4h[0], c * 512, start=(c < 2))
            # h0 gi1, gi2 at subtile cadence
            for gi in (1, 2):
                for ibl in range(2):
                    block8(gi * 2 + ibl, etile[(0, gi)], ibl)
            for gi in range(3, 8):
                for ibl in range(2):
                    block8(gi * 2 + ibl, etile[(0, gi)], ibl)
            # h1: E4 parts then gi1..gi5
            for ibl in range(2):
                block8(16 + ibl, e4h[1], ibl)
            for gi in range(1, 6):
                for ibl in range(2):
                    block8(16 + gi * 2 + ibl, etile[(1, gi)], ibl)
            # tail: gi6, gi7 swept (ot, bc)-outer so the 8 banks close
            # staggered ~850ns apart and drains/DMAs chase them
            def tail_mms(ot, bc):
                for gi in (6, 7):
                    for ibl in range(2):
                        pos = 16 + gi * 2 + ibl
                        mm(pos, ot, bc, etile[(1, gi)],
                           ibl * B_SH + bc * 512,
                           stop=(gi == 7 and ibl == 1))

            # ---- drain psum -> SBUF (bf16) -> per-piece DMA out ---------
            # output pushes spread over Pool (bc0 banks), ACT (bc1 banks +
            # final piece) and SP (piece A) so the last pieces hit engines
            # that are free at closure time; bf16 halves the output wire
            pool_ops = []
            sp_out = []

            def drain(ot, bc, lo, hi, copy_eng, dma_eng):
                osb = ob_pool.tile([128, hi - lo], mybir.dt.bfloat16,
                                   tag="ob", name=f"ob{ot}_{bc}_{lo}")
                dst = out_d[ot * 128:(ot + 1) * 128,
                            bc * 512 + lo:bc * 512 + hi]
                src = psum[ot][bc][:, lo:hi]
                if copy_eng == "dve":
                    cp = nc.vector.tensor_copy(osb[:], src)
                    dve_ops.append(cp)
                else:
                    cp = nc.scalar.activation(osb[:], src, AF.Copy)
                    act_ops.append(cp)
                if dma_eng == "pool":
                    pool_ops.append(nc.gpsimd.dma_start(dst, osb[:]))
                elif dma_eng == "sp":
                    sp_out.append(nc.sync.dma_start(dst, osb[:]))
                else:
                    # branch off the copy, NOT into the act chain: a compute
                    # op pinned after a dma_start waits for the full DMA
                    # completion semaphore (~wire + 900ns)
                    dma = nc.scalar.dma_start(dst, osb[:])
                    add_dep_helper(dma.ins, cp.ins, sync=False,
                                   reason="act out dma after its copy")

            for ot in range(N_OT):
                for bc in range(N_BC):
                    tail_mms(ot, bc)
                    if bc == 0:
                        drain(ot, bc, 0, 512, "dve", "pool")
                    elif ot == N_OT - 1:
                        # last-closing bank: copy and DMA both on ACT,
                        # which is guaranteed free at closure
                        drain(ot, bc, 0, 512, "act", "act")
                    else:
                        drain(ot, bc, 0, 512, "act", "sp")

            # ---- pin engine orders --------------------------------------
            def pin(ops, reason):
                for a, b in zip(ops, ops[1:]):
                    add_dep_helper(b.ins, a.ins, sync=False, reason=reason)

            pin(act_ops, "act order")
            pin(dve_ops, "dve order")
            pin(pe_ops, "pe order")
            pin(dma_chain + sp_out, "sp dma order")
            pin(pool_ops, "pool dma order")
    nc.compile()
    return nc


def get_nc():
    if "nc" not in _NC_CACHE:
        _NC_CACHE["nc"] = build_nc()
    return _NC_CACHE["nc"]


def prep_inputs(x, weights, coefficients):
    x = np.asarray(x, dtype=np.float32)
    weights = np.asarray(weights, dtype=np.float32)
    coefficients = np.asarray(coefficients, dtype=np.float32)
    # W2T[k=(g,i), o] = coeff[o,i,g] * W[o,i] * exp(-a*(c_g^2 - c4^2))
    w2 = coefficients * weights[:, :, None]                   # [O, I, G]
    fold = np.exp(-ALPHA * (CENTERS.astype(np.float64) ** 2 - C4 * C4))
    w2t = w2.transpose(2, 1, 0).astype(np.float64) * fold[:, None, None]
    # reorder source tiles (g, ib) into device positions (h, gi, ib_loc)
    src = np.ascontiguousarray(w2t).reshape(G, N_IBLK, 128, O_SZ)
    dev = np.empty((2, G, 2, 128, O_SZ), np.float32)
    for h in range(2):
        for gi in range(G):
            for ibl in range(2):
                dev[h, gi, ibl] = src[GIDX[gi], 2 * h + ibl]
    import ml_dtypes
    w2t = np.ascontiguousarray(
        dev.reshape(K, O_SZ).astype(ml_dtypes.bfloat16))
    xT = np.ascontiguousarray(x.T)  # [I, B]
    in_maps = [
        {
            "xt": np.ascontiguousarray(xT[:, c * B_SH:(c + 1) * B_SH]),
            "w2t": w2t,
        }
        for c in range(NCORES)
    ]
    return in_maps


def kernel(x, weights, coefficients):
    nc = get_nc()
    in_maps = prep_inputs(x, weights, coefficients)
    res = run_bass_kernel_spmd(nc, in_maps, core_ids=list(range(NCORES)))
    out = np.empty((B, O_SZ), dtype=np.float32)
    for c in range(NCORES):
        out[c * B_SH:(c + 1) * B_SH, :] = (
            np.asarray(res.results[c]["out_t"]).astype(np.float32).T)
    return out


# revision 59
# speedup vs baseline: 1.0061x; 1.0022x over previous
"""CustomGaussianLayer Trainium2 kernel.

Math: out[b,o] = sum_{i,g} exp(-0.5*((tanh(x[b,i])-c_g)/w)^2) * coeff[o,i,g]*W[o,i]
 == E @ W2T  with  E[b, k=(g,i)] Gaussian basis,  W2T[k, o] folded weights.

Reference-center factorization (c4 = centers[4] = 1/7, smallest |c|):
  E_4 = exp(-a*(t-c4)^2)            directly: (t-c4)^2 -> Exp(-a*.)
  E_g = E_4 * exp(2a*(c_g-c4)*t)    for g != 4 (one ACT exp + one DVE mult)
Host folds exp(-a*(c_g^2-c4^2)) <= 1 into the weights (no overflow: c4 is
the smallest center), replacing the exp(-a t^2)*exp(2a c_g t) pair per g.

Per core (data-parallel over batch, 1024 rows each):
  ACT: tanh + exps (+ h1 squares) ; DVE: h0 sub/sq + bf16 E mults (2x
  mode) ; PE: bf16 matmuls [o,b] += W2T_k^T E_k (1 cycle/row). Startup is
  512-col fine-grained with the input DMA chain in consumer order; PE
  ramps on warm-up matmuls against a zeroed scratch from ~1us. The last
  4 k-tiles are swept (ot,bc)-outer so the 8 psum banks close staggered
  ~850ns apart; bf16 drains chase on DVE/ACT with their DMAs spread over
  Pool/SP/ACT queues so the output wire and pushes stay off the critical
  path.
"""

import numpy as np

import concourse.bacc as bacc
import concourse.mybir as mybir
import concourse.tile as tile
from concourse.bass_utils import run_bass_kernel_spmd
from concourse.tile import add_dep_helper

G = 8
I_SZ = 512
O_SZ = 512
B = 8192
NCORES = 8
B_SH = B // NCORES          # 1024 batch rows per core
K = I_SZ * G                # 4096 contraction
N_IBLK = I_SZ // 128        # 4 partition blocks of i
FREE = N_IBLK * B_SH        # 4096 free layout (i_blk, b)
HALF = FREE // 2            # 2048 (i_blk 0-1 | 2-3)
N_OT = O_SZ // 128          # 4 output tiles
N_BC = B_SH // 512          # 2 batch chunks of 512 (psum free limit fp32)

ALPHA = 24.5                # 0.5 / width^2, width = 1/7
N_WARMUP = 8
CENTERS = np.linspace(-1.0, 1.0, G).astype(np.float32)
C4 = float(CENTERS[4])      # 1/7, reference center
GIDX = [4, 0, 1, 2, 3, 5, 6, 7]   # E-tile production order -> g
# device k-tile position: pos = h*16 + gi*2 + ib_loc

F32 = mybir.dt.float32
F32R = mybir.dt.float32r
BF16 = mybir.dt.bfloat16
AF = mybir.ActivationFunctionType
ALU = mybir.AluOpType

_NC_CACHE = {}


def build_nc():
    nc = bacc.Bacc("TRN2", target_bir_lowering=False)
    xt_d = nc.dram_tensor("xt", [I_SZ, B_SH], F32, kind="ExternalInput")
    w2t_d = nc.dram_tensor("w2t", [K, O_SZ], BF16, kind="ExternalInput")
    out_d = nc.dram_tensor(
        "out_t", [O_SZ, B_SH], mybir.dt.bfloat16, kind="ExternalOutput")

    with tile.TileContext(nc) as tc:
        with (
            tc.tile_pool(name="w2", bufs=1) as w2_pool,
            tc.tile_pool(name="xt", bufs=1) as xt_pool,
            tc.tile_pool(name="tt", bufs=1) as tt_pool,
            tc.tile_pool(name="uu", bufs=2) as uu_pool,
            tc.tile_pool(name="qq", bufs=2) as qq_pool,
            tc.tile_pool(name="e4", bufs=2) as e4_pool,
            tc.tile_pool(name="bb", bufs=3) as bb_pool,
            tc.tile_pool(name="ee", bufs=4) as ee_pool,
            tc.tile_pool(name="wm", bufs=1) as wm_pool,
            tc.tile_pool(name="ps", bufs=1, space="PSUM") as ps_pool,
            tc.tile_pool(name="ob", bufs=8) as ob_pool,
        ):
            act_ops = []   # pinned ACT engine order
            dve_ops = []   # pinned DVE engine order
            pe_ops = []    # pinned PE engine order

            # trigger the ACT table load immediately (~1.3us; the
            # exp_and_others set serves Exp, Tanh and Square together)
            actwarm = wm_pool.tile([128, 1], F32, tag="actwarm")
            act_ops.append(nc.scalar.activation(
                actwarm[:], nc.const_aps.tensor(0.0, (128, 1)), AF.Exp,
            ))

            # zeroed scratch for PE warm-up matmuls: lets the PE p-state
            # ramp start at ~1us instead of waiting for the first w2 DMA
            warm = wm_pool.tile([128, 640], BF16, tag="warm")
            dve_ops.append(nc.vector.memset(
                warm[:].bitcast(mybir.dt.uint16), 0))
            # per-partition scalar bias (-c4) for the h1 Square activations
            c4b = wm_pool.tile([128, 1], F32, tag="c4b")
            dve_ops.append(nc.vector.memset(c4b[:], -C4))

            w2_all = w2_pool.tile([128, (K // 128) * O_SZ], BF16, tag="w2all")
            w2t_v = w2t_d[:, :].rearrange("(kt p) o -> p kt o", p=128)
            xt_sb = xt_pool.tile([128, FREE], F32, tag="xt")
            tt = tt_pool.tile([128, FREE], F32, tag="tt")
            xt_v = xt_d[:, :].rearrange("(ib p) b -> p ib b", p=128)

            def w2_dma(kt_lo, kt_hi):
                return nc.sync.dma_start(
                    w2_all[:, kt_lo * O_SZ:kt_hi * O_SZ]
                    .rearrange("p (kt o) -> p kt o", o=O_SZ),
                    w2t_v[:, kt_lo:kt_hi, :],
                )

            def xt_dma_fine(c):
                # one 512-col chunk (c in units of 512 free cols)
                return nc.sync.dma_start(
                    xt_sb[:, c * 512:(c + 1) * 512],
                    xt_v[:, c // 2, (c % 2) * 512:(c % 2 + 1) * 512],
                )

            def xt_dma_q(p):
                # one 256-col piece (p in units of 256 free cols)
                return nc.sync.dma_start(
                    xt_sb[:, p * 256:(p + 1) * 256],
                    xt_v[:, p // 4, (p % 4) * 256:(p % 4 + 1) * 256],
                )

            def xt_dma(ib_lo, ib_hi):
                return nc.sync.dma_start(
                    xt_sb[:, ib_lo * B_SH:ib_hi * B_SH]
                    .rearrange("p (ib b) -> p ib b", b=B_SH),
                    xt_v[:, ib_lo:ib_hi, :],
                )

            # SP DMA lane in consumer order; xt chunks lead so the ACT
            # tanh chain never stalls, w2 pos-pairs chase PE consumption
            dma_chain = [
                xt_dma_fine(0),        # c0: ib0 b0-511  -> tanh_c0
                xt_dma_q(2),           # c1 first half: tanh_c1a sooner
                xt_dma_q(3),           # c1 second half
                xt_dma_fine(2),        # c2
                w2_dma(0, 1),          # pos0 (h0 gi0=g4 ib0): first real mm
                xt_dma_fine(3),        # c3
                w2_dma(1, 2),          # pos1
                w2_dma(2, 4),          # pos2-3 (gi1)
                w2_dma(4, 8),          # pos4-7 (gi2-3)
                w2_dma(8, 12),         # pos8-11 (gi4-5 h0)
                xt_dma(2, 4),          # h1
                w2_dma(12, 16),        # pos12-15 (gi6-7 h0)
                w2_dma(16, 24),        # h1 gi0-3
                w2_dma(24, 32),        # h1 gi4-7
            ]

            psum = [
                [
                    ps_pool.tile(
                        [128, 512], F32,
                        name=f"ps{ot}_{bc}", tag=f"ps{ot}_{bc}",
                    )
                    for bc in range(N_BC)
                ]
                for ot in range(N_OT)
            ]

            # PE warm-up on the zeroed scratch (disjoint lhsT/rhs regions)
            for w in range(N_WARMUP):
                pe_ops.append(nc.tensor.matmul(
                    psum[0][0][:], warm[:, 0:128], warm[:, 128:640],
                    start=(w == 0), stop=(w == N_WARMUP - 1),
                ))

            # ---- basis production ---------------------------------------
            s_g = {g: float(2.0 * ALPHA * (CENTERS[g] - C4)) for g in range(G)}
            e4h = [
                e4_pool.tile([128, HALF], BF16, tag="e4", name=f"e4h{h}")
                for h in range(2)
            ]

            # h0 fine phase (512-col chunks): tanh + E4-exp on ACT,
            # (t-c4) and square on DVE; t1 split in 256-col halves so the
            # first half starts as soon as its smaller DMA lands
            t_f, e4_f = [], []
            for c in range(4):
                sl = slice(c * 512, (c + 1) * 512)
                if c == 1:
                    t_f.append([
                        nc.scalar.activation(
                            tt[:, 512:768], xt_sb[:, 512:768], AF.Tanh),
                        nc.scalar.activation(
                            tt[:, 768:1024], xt_sb[:, 768:1024], AF.Tanh),
                    ])
                else:
                    t_f.append(nc.scalar.activation(
                        tt[:, sl], xt_sb[:, sl], AF.Tanh))
                if c == 0:
                    # 256-col halves: the a-half sub/sq finishes during the
                    # t1 ops, so E4_0a slots right after t1b instead of
                    # waiting ~325ns on the full-width DVE chain
                    e4_pair = []
                    for hl in range(2):
                        hsl = slice(hl * 256, (hl + 1) * 256)
                        u = uu_pool.tile([128, 256], F32, tag="uf")
                        q = qq_pool.tile([128, 256], F32, tag="qf")
                        dve_ops.append(nc.vector.tensor_scalar_sub(
                            u[:], tt[:, hsl], C4))
                        dve_ops.append(nc.vector.tensor_tensor(
                            q[:], u[:], u[:], op=ALU.mult))
                        e4_pair.append(nc.scalar.activation(
                            e4h[0][:, hsl], q[:], AF.Exp, scale=-ALPHA))
                    e4_f.append(e4_pair)
                    continue
                u = uu_pool.tile([128, 512], F32, tag="uf")
                q = qq_pool.tile([128, 512], F32, tag="qf")
                dve_ops.append(nc.vector.tensor_scalar_sub(u[:], tt[:, sl], C4))
                dve_ops.append(nc.vector.tensor_tensor(q[:], u[:], u[:], op=ALU.mult))
                e4_f.append(nc.scalar.activation(
                    e4h[0][:, sl], q[:], AF.Exp, scale=-ALPHA))
            # B' / E tiles for gi >= 1
            btile = {}
            etile = {}

            def b_part(h, gi, lo, hi):
                g = GIDX[gi]
                if (h, gi) not in btile:
                    btile[(h, gi)] = bb_pool.tile(
                        [128, HALF], BF16, tag="bb", name=f"b_{h}_{gi}")
                bt = btile[(h, gi)]
                hsl = h * HALF
                act_ops.append(nc.scalar.activation(
                    bt[:, lo:hi], tt[:, hsl + lo:hsl + hi], AF.Exp,
                    scale=s_g[g]))

            def make_b(h, gi, parts):
                for lo, hi in parts:
                    b_part(h, gi, lo, hi)

            def make_e(h, gi, parts):
                et = ee_pool.tile([128, HALF], BF16, tag="ee",
                                  name=f"e_{h}_{gi}")
                etile[(h, gi)] = et
                bt = btile[(h, gi)]
                for lo, hi in parts:
                    dve_ops.append(nc.vector.tensor_tensor(
                        et[:, lo:hi], e4h[h][:, lo:hi], bt[:, lo:hi],
                        op=ALU.mult))

            SUB = [(0, 1024), (1024, 2048)]
            FULLP = [(0, 2048)]
            # h0 fine-phase ACT order: t0, t1, E4_0, t2, E4_1, t3, B1a,
            # E4_2, E4_3, ... — B1a needs only t0/t1 + E4_0/E4_1 downstream,
            # so hoisting it lets the gi1 feed (PE saturation anchor) start
            # ~1.3us earlier while c2/c3 stall on E4_2/E4_3 anyway
            act_ops += [t_f[0], t_f[1][0], t_f[1][1], e4_f[0][0], e4_f[0][1],
                        t_f[2], e4_f[1], t_f[3], e4_f[2]]
            b_part(0, 1, 0, 512)                      # B1 quarter 0
            act_ops += [e4_f[3]]
            b_part(0, 1, 512, 1024)                   # B1 quarter 1
            b_part(0, 1, 1024, 2048)                  # B1b
            # h0: gi2 at 1024-col subtiles; gi3..7 full-half
            make_b(0, 2, SUB)
            for gi in range(3, 8):
                make_b(0, gi, FULLP)
            # h1 E4 chain at 1024-col parts: tanh, Square(t - c4), exp
            t_h1, e4_h1 = [], []
            q_h1 = []
            for p in range(2):
                sl = slice(HALF + p * 1024, HALF + (p + 1) * 1024)
                esl = slice(p * 1024, (p + 1) * 1024)
                t_i = nc.scalar.activation(tt[:, sl], xt_sb[:, sl], AF.Tanh)
                q = qq_pool.tile([128, 1024], F32, tag="qc")
                q_i = nc.scalar.activation(q[:], tt[:, sl], AF.Square, bias=c4b[:])
                e_i = nc.scalar.activation(
                    e4h[1][:, esl], q[:], AF.Exp, scale=-ALPHA)
                t_h1.append(t_i); q_h1.append(q_i); e4_h1.append(e_i)
            act_ops += [t_h1[0], q_h1[0], e4_h1[0],
                        t_h1[1], q_h1[1], e4_h1[1]]
            for gi in range(1, 8):
                make_b(1, gi, FULLP)

            # DVE E-mult order (matches ACT production order; gi1-ib0 at
            # 512-col quarters so the first gi1 matmuls feed right after
            # the B1 quarters land)
            make_e(0, 1, [(0, 512), (512, 1024), (1024, 2048)])
            make_e(0, 2, SUB)
            for gi in range(3, 8):
                make_e(0, gi, FULLP)
            for gi in range(1, 8):
                make_e(1, gi, FULLP)

            # ---- PE: matmuls in consumption order -----------------------
            def mm(pos, ot, bc, et, ebase, start=False, stop=False):
                lhsT = w2_all[:, pos * O_SZ + ot * 128: pos * O_SZ + (ot + 1) * 128]
                pe_ops.append(nc.tensor.matmul(
                    psum[ot][bc][:], lhsT, et[:, ebase:ebase + 512],
                    start=start, stop=stop))

            def block8(pos, et, ibl):
                for ot in range(N_OT):
                    for bc in range(N_BC):
                        mm(pos, ot, bc, et, ibl * B_SH + bc * 512)

            # h0 E4 fine chunks: chunk c -> (pos=c//2, bc=c%2)
            for c in range(4):
                for ot in range(N_OT):
                    mm(c // 2, ot, c % 2, e4h[0], c * 512, start=(c < 2))
            # h0 gi1, gi2 at subtile cadence
            for gi in (1, 2):
                for ibl in range(2):
                    block8(gi * 2 + ibl, etile[(0, gi)], ibl)
            for gi in range(3, 8):
                for ibl in range(2):
                    block8(gi * 2 + ibl, etile[(0, gi)], ibl)
            # h1: E4 parts then gi1..gi5
            for ibl in range(2):
                block8(16 + ibl, e4h[1], ibl)
            for gi in range(1, 6):
                for ibl in range(2):
                    block8(16 + gi * 2 + ibl, etile[(1, gi)], ibl)
            # tail: gi6, gi7 swept (ot, bc)-outer so the 8 banks close
            # staggered ~850ns apart and drains/DMAs chase them
            def tail_mms(ot, bc):
                for gi in (6, 7):
                    for ibl in range(2):
                        pos = 16 + gi * 2 + ibl
                        mm(pos, ot, bc, etile[(1, gi)],
                           ibl * B_SH + bc * 512,
                           stop=(gi == 7 and ibl == 1))

            # ---- drain psum -> SBUF (bf16) -> per-piece DMA out ---------
            # output pushes spread over Pool (bc0 banks), ACT (bc1 banks +
            # final piece) and SP (piece A) so the last pieces hit engines
            # that are free at closure time; bf16 halves the output wire
            pool_ops = []
            sp_out = []

            def drain(ot, bc, lo, hi, copy_eng, dma_eng):
                osb = ob_pool.tile([128, hi - lo], mybir.dt.bfloat16,
                                   tag="ob", name=f"ob{ot}_{bc}_{lo}")
                dst = out_d[ot * 128:(ot + 1) * 128,
                            bc * 512 + lo:bc * 512 + hi]
                src = psum[ot][bc][:, lo:hi]
                if copy_eng == "dve":
                    cp = nc.vector.tensor_copy(osb[:], src)
                    dve_ops.append(cp)
                else:
                    cp = nc.scalar.activation(osb[:], src, AF.Copy)
                    act_ops.append(cp)
                if dma_eng == "pool":
                    pool_ops.append(nc.gpsimd.dma_start(dst, osb[:]))
                elif dma_eng == "sp":
                    sp_out.append(nc.sync.dma_start(dst, osb[:]))
                else:
                    # branch off the copy, NOT into the act chain: a compute
                    # op pinned after a dma_start waits for the full DMA
                    # completion semaphore (~wire + 900ns)
                    dma = nc.scalar.dma_start(dst, osb[:])
                    add_dep_helper(dma.ins, cp.ins, sync=False,
                                   reason="act out dma after its copy")

            for ot in range(N_OT):
                for bc in range(N_BC):
                    tail_mms(ot, bc)
                    if bc == 0:
                        drain(ot, bc, 0, 512, "dve", "pool")
                    elif ot == N_OT - 1:
                        # last-closing bank: copy and DMA both on ACT,
                        # which is guaranteed free at closure
                        drain(ot, bc, 0, 512, "act", "act")
                    else:
                        drain(ot, bc, 0, 512, "act", "sp")

            # ---- pin engine orders --------------------------------------
            def pin(ops, reason):
                for a, b in zip(ops, ops[1:]):
                    add_dep_helper(b.ins, a.ins, sync=False, reason=reason)

            pin(act_ops, "act order")
            pin(dve_ops, "dve order")
            pin(pe_ops, "pe order")
            pin(dma_chain + sp_out, "sp dma order")
            pin(pool_ops, "pool dma order")
    nc.compile()
    return nc


def get_nc():
    if "nc" not in _NC_CACHE:
        _NC_CACHE["nc"] = build_nc()
    return _NC_CACHE["nc"]


def prep_inputs(x, weights, coefficients):
    x = np.asarray(x, dtype=np.float32)
    weights = np.asarray(weights, dtype=np.float32)
    coefficients = np.asarray(coefficients, dtype=np.float32)
    # W2T[k=(g,i), o] = coeff[o,i,g] * W[o,i] * exp(-a*(c_g^2 - c4^2))
    w2 = coefficients * weights[:, :, None]                   # [O, I, G]
    fold = np.exp(-ALPHA * (CENTERS.astype(np.float64) ** 2 - C4 * C4))
    w2t = w2.transpose(2, 1, 0).astype(np.float64) * fold[:, None, None]
    # reorder source tiles (g, ib) into device positions (h, gi, ib_loc)
    src = np.ascontiguousarray(w2t).reshape(G, N_IBLK, 128, O_SZ)
    dev = np.empty((2, G, 2, 128, O_SZ), np.float32)
    for h in range(2):
        for gi in range(G):
            for ibl in range(2):
                dev[h, gi, ibl] = src[GIDX[gi], 2 * h + ibl]
    import ml_dtypes
    w2t = np.ascontiguousarray(
        dev.reshape(K, O_SZ).astype(ml_dtypes.bfloat16))
    xT = np.ascontiguousarray(x.T)  # [I, B]
    in_maps = [
        {
            "xt": np.ascontiguousarray(xT[:, c * B_SH:(c + 1) * B_SH]),
            "w2t": w2t,
        }
        for c in range(NCORES)
    ]
    return in_maps


def kernel(x, weights, coefficients):
    nc = get_nc()
    in_maps = prep_inputs(x, weights, coefficients)
    res = run_bass_kernel_spmd(nc, in_maps, core_ids=list(range(NCORES)))
    out = np.empty((B, O_SZ), dtype=np.float32)
    for c in range(NCORES):
        out[c * B_SH:(c + 1) * B_SH, :] = (
            np.asarray(res.results[c]["out_t"]).astype(np.float32).T)
    return out
